# revision 5
# baseline (speedup 1.0000x reference)
"""Trainium2 Bass kernel for nn_MaskedSelfAttention (causal, QK rms-norm) — v2.

Sharding: 8 cores = 2 (batch) x 4 (head groups of 4 heads). Host sums the
4 partial FC outputs per batch.

v2 redesign vs v1:
- e-major QKV for q,k (out [e, l]): qT/kT produced directly, no PE transposes.
- RMS sumsq via a replication matmul (onesrep block-diag lhsT) so the
  1/rms factors land partition-replicated; no transposes / broadcasts.
- V carries 64 extra all-ones columns so the PV matmul emits the softmax
  denominator replicated across partitions 64..127; normalization is then
  one DVE reciprocal + one DVE mult per (hp, c, h2).
- bf16 matmul operands everywhere (psum accumulation stays fp32).
- Phases interleaved per 512-query chunk c: A(c) -> B(hp=0,c) -> norm ->
  B(hp=1,c) -> norm -> FC(c), so exp (ACT-bound) overlaps PE work and the
  output DMA streams throughout.
"""

import numpy as np
import ml_dtypes

import concourse.bacc as bacc
import concourse.mybir as mybir
import concourse.tile as tile
from concourse.bass_utils import run_bass_kernel_spmd

B, L, D = 2, 2048, 1024
DH = 64
NH = D // DH            # 16 heads total
P = 128
NHC = 4                 # heads per core
LB = L // P             # 16 l-blocks
KB = D // P             # 8 contraction blocks
NC4 = 4                 # query chunks of 512
EPS = 1e-5
F32 = mybir.dt.float32
F32R = mybir.dt.float32r
BF16 = mybir.dt.bfloat16
MULT = mybir.AluOpType.mult
FX = mybir.ActivationFunctionType

bf16 = ml_dtypes.bfloat16

_CACHE = {}


def R(ap):
    return ap.bitcast(F32R)


def _build_nc():
    nc = bacc.Bacc("TRN2", target_bir_lowering=False, debug=False)

    xT = nc.dram_tensor("xT", (D, L), BF16, kind="ExternalInput").ap()
    # e-order columns: [q-hp0 | k-hp0 | q-hp1 | k-hp1] (128 each: h2=0 dh64,
    # h2=1 dh64) then v (256: head 2*hp+h2, dh). k columns pre-scaled by
    # wqk = q_norm_w * k_norm_w.
    wqkv = nc.dram_tensor("wqkv", (D, 6 * P), BF16, kind="ExternalInput").ap()
    wfcT = nc.dram_tensor("wfcT", (2 * P, D), BF16, kind="ExternalInput").ap()
    # [q-block | k-block]; block-diag 64x64 ones; k block weighted 1/wqk^2
    onesrep = nc.dram_tensor("onesrep", (P, 2 * P), F32, kind="ExternalInput").ap()
    triu = nc.dram_tensor("triu", (P, P), BF16, kind="ExternalInput").ap()
    outp = nc.dram_tensor("outp", (L, D), F32, kind="ExternalOutput").ap()

    xT_r = xT.rearrange("(ko p) l -> p ko l", p=P)

    with tile.TileContext(nc) as tc:
        with (
            tc.tile_pool(name="cpool", bufs=1) as cpool,
            tc.tile_pool(name="wpool", bufs=1) as wpool,
            tc.tile_pool(name="xpool", bufs=2) as xpool,
            tc.tile_pool(name="ppool", bufs=1) as ppool,
            tc.tile_pool(name="work", bufs=2) as work,
            tc.tile_pool(name="ptpool", bufs=4) as ptpool,
            tc.tile_pool(name="otpool", bufs=3) as otpool,
            tc.tile_pool(name="psA", bufs=1, space="PSUM") as psA,
            tc.tile_pool(name="psB", bufs=1, space="PSUM") as psB,
        ):
            # ---- x chunk 0 + weights first (first-matmul critical path) ----
            xpool_xc0 = xpool.tile([P, KB, 512], BF16, tag="xc", name="xc_0")
            w_sb = wpool.tile([P, KB, 6 * P], BF16)
            wqkv_r = wqkv.rearrange("(ko p) e -> p ko e", p=P)
            for h in range(2):
                kx = slice(h * 4, (h + 1) * 4)
                nc.sync.dma_start(w_sb[:, kx, :], wqkv_r[:, kx, :])
                nc.sync.dma_start(xpool_xc0[:, kx, :], xT_r[:, kx, 0:512])
            onesrep_sb = cpool.tile([P, 2, P], F32R)
            nc.sync.dma_start(onesrep_sb, R(onesrep.rearrange("p (a q) -> p a q", q=P)))
            triu_sb = cpool.tile([P, P], BF16)
            nc.sync.dma_start(triu_sb, triu)
            biasq = cpool.tile([P, 1], F32)
            nc.vector.memset(biasq, DH * EPS)
            wfc_sb = wpool.tile([P, 2, D], BF16)

            # ---- persistent activations ----
            qT = ppool.tile([P, 2, L], BF16)          # [h2*64+dh, hp, l]
            kT = ppool.tile([P, 2, L], BF16)
            vext = ppool.tile([P, LB, NHC, P], BF16)  # cols DH..P-1 = ones
            oT = ppool.tile([P, 2, L], BF16)
            nc.gpsimd.memset(vext[:, :, :, DH:P], 1.0)

            xcs = {}

            def emit_A(c, ebs=(0, 1, 2, 3)):
                # x chunk DMAs: prefetch next chunk; c=0 chunk requested at head
                if 0 in ebs and c + 1 < NC4:
                    xc2 = xpool.tile([P, KB, 512], BF16, tag="xc", name=f"xc_{c+1}")
                    nc.sync.dma_start(xc2, xT_r[:, :, (c + 1) * 512 : (c + 2) * 512])
                    xcs[c + 1] = xc2
                if c == 0 and 0 in ebs:
                    nc.sync.dma_start(wfc_sb, wfcT.rearrange("(g p) e -> p g e", p=P))
                xc = xcs[c]

                def emit_v(mi):
                    m = 4 * c + mi
                    psv = psA.tile([P, 256], F32, tag="mm", bufs=2, name=f"v_{m}")
                    for k in range(KB):
                        nc.tensor.matmul(
                            psv,
                            lhsT=xc[:, k, mi * P : (mi + 1) * P],
                            rhs=w_sb[:, k, 4 * P : 6 * P],
                            start=(k == 0),
                            stop=(k == KB - 1),
                        )
                    nc.vector.tensor_copy(
                        vext[:, m, :, 0:DH], psv.rearrange("p (h d) -> p h d", d=DH)
                    )

                for eb in ebs:  # q-hp0, k-hp0, q-hp1, k-hp1
                    hp, is_k = eb // 2, eb % 2
                    ps = psA.tile([P, 512], F32, tag="mm", bufs=2, name=f"qk_{c}_{eb}")
                    for k in range(KB):
                        nc.tensor.matmul(
                            ps,
                            lhsT=w_sb[:, k, eb * P : (eb + 1) * P],
                            rhs=xc[:, k, :],
                            start=(k == 0),
                            stop=(k == KB - 1),
                        )
                    sq = work.tile([P, 512], F32R, tag="sq", name=f"sq_{c}_{eb}")
                    nc.scalar.activation(sq, ps, FX.Square)
                    ssq = psB.tile([P, 512], F32, tag="st", bufs=2, name=f"ssq_{c}_{eb}")
                    nc.tensor.matmul(
                        ssq, lhsT=onesrep_sb[:, is_k, :], rhs=sq, start=True, stop=True
                    )
                    rin = work.tile([P, 512], F32, tag="rin", name=f"rin_{c}_{eb}")
                    nc.scalar.activation(rin, ssq, FX.Sqrt, bias=biasq[:, :], scale=1.0)
                    inv = work.tile([P, 512], F32, tag="inv", name=f"inv_{c}_{eb}")
                    nc.vector.reciprocal_approx_fast(inv, rin)
                    dst = kT if is_k else qT
                    nc.vector.tensor_tensor(
                        dst[:, hp, c * 512 : (c + 1) * 512], ps, inv, MULT
                    )
                    if eb == 1:
                        emit_v(0)
                        emit_v(1)
                    elif eb == 3:
                        emit_v(2)
                        emit_v(3)

            def emit_B(hp, c):
                oTps = [
                    psA.tile([P, 512], F32, tag=f"oT{h2}", bufs=1, name=f"oT_{hp}_{c}_{h2}")
                    for h2 in range(2)
                ]
                nj = 4 * c + 4
                for j in range(nj):
                    off = max(0, j * P - c * 512)
                    W = 512 - off
                    st = psB.tile([P, 2, 512], F32, tag="st", bufs=2, name=f"st_{hp}_{c}_{j}")
                    for h2 in range(2):
                        nc.tensor.matmul(
                            st[:, h2, 0:W],
                            lhsT=kT[h2 * DH : (h2 + 1) * DH, hp, j * P : (j + 1) * P],
                            rhs=qT[h2 * DH : (h2 + 1) * DH, hp, c * 512 + off : (c + 1) * 512],
                            start=True,
                            stop=True,
                        )
                    pt = ptpool.tile([P, 2, 512], BF16, tag="pt", name=f"pt_{hp}_{c}_{j}")
                    nc.scalar.activation(pt[:, :, 0:W], st[:, :, 0:W], FX.Exp, scale=8.0)
                    if j >= 4 * c:
                        nc.vector.tensor_tensor(
                            pt[:, :, 0:P],
                            pt[:, :, 0:P],
                            triu_sb[:, None, :].to_broadcast((P, 2, P)),
                            MULT,
                        )
                    for h2 in range(2):
                        nc.tensor.matmul(
                            oTps[h2][:, off:512],
                            lhsT=vext[:, j, 2 * hp + h2, :],
                            rhs=pt[:, h2, 0:W],
                            start=(j == 0),
                            stop=(j == nj - 1),
                            skip_group_check=True,
                        )
                # normalize: denominator sits replicated in partitions 64..127
                for h2 in range(2):
                    rbd = work.tile([DH, 512], F32, tag="rbd", name=f"rbd_{hp}_{c}_{h2}")
                    nc.vector.tensor_copy(rbd, oTps[h2][DH:P, :])
                    rb = work.tile([DH, 512], F32, tag="rb", name=f"rb_{hp}_{c}_{h2}")
                    nc.vector.reciprocal_approx_fast(rb, rbd)
                    nc.vector.tensor_tensor(
                        oT[h2 * DH : (h2 + 1) * DH, hp, c * 512 : (c + 1) * 512],
                        oTps[h2][0:DH, :],
                        rb,
                        MULT,
                    )

            def emit_FC(c):
                for mi in range(4):
                    m = 4 * c + mi
                    ot = otpool.tile([P, 2, 512], F32, tag="ot", name=f"ot_{m}")
                    for n in range(2):
                        fp = psA.tile([P, 512], F32, tag="mm", bufs=2, name=f"fc_{m}_{n}")
                        for g in range(2):
                            nc.tensor.matmul(
                                fp,
                                lhsT=oT[:, g, m * P : (m + 1) * P],
                                rhs=wfc_sb[:, g, n * 512 : (n + 1) * 512],
                                start=(g == 0),
                                stop=(g == 1),
                            )
                        nc.vector.tensor_copy(ot[:, n, :], fp)
                    nc.sync.dma_start(
                        outp[m * P : (m + 1) * P, :].rearrange("p (n e) -> p n e", n=2), ot
                    )

            xcs[0] = xpool_xc0
            emit_A(0)
            for c in range(NC4):
                emit_B(0, c)
                if c + 1 < NC4:
                    emit_A(c + 1, ebs=(0, 1))
                emit_B(1, c)
                if c + 1 < NC4:
                    emit_A(c + 1, ebs=(2, 3))
                emit_FC(c)

    nc.compile()
    return nc


def _make_in_maps(x, w_qkv, w_fc, q_norm_w, k_norm_w):
    wqk = (q_norm_w.astype(np.float64) * k_norm_w.astype(np.float64))  # (64,)
    triu_f = np.triu(np.ones((P, P), dtype=np.float32)).astype(bf16)
    ind = (np.arange(P)[:, None] // DH) == (np.arange(P)[None, :] // DH)
    onesrep = np.zeros((P, 2 * P), dtype=np.float32)
    onesrep[:, 0:P] = ind.astype(np.float32)
    onesrep[:, P : 2 * P] = ind.astype(np.float64) / np.tile(wqk, 2)[:, None] ** 2

    wqkv_e = {}
    wfcTs = {}
    for hg in range(4):
        cols = []
        for eb in range(4):
            hp, is_k = eb // 2, eb % 2
            for h2 in range(2):
                g_h = hg * NHC + 2 * hp + h2
                blk = w_qkv[is_k * D + g_h * DH : is_k * D + (g_h + 1) * DH, :]
                if is_k:
                    blk = blk * wqk[:, None]
                cols.append(blk)
        for h in range(NHC):
            g_h = hg * NHC + h
            cols.append(w_qkv[2 * D + g_h * DH : 2 * D + (g_h + 1) * DH, :])
        wqkv_e[hg] = np.ascontiguousarray(np.concatenate(cols, axis=0).T).astype(bf16)
        wfcTs[hg] = np.ascontiguousarray(
            w_fc.T[hg * NHC * DH : (hg + 1) * NHC * DH]
        ).astype(bf16)
    xTs = [np.ascontiguousarray(x[b].T).astype(bf16) for b in range(B)]
    in_maps = []
    for core in range(8):
        b, hg = core // 4, core % 4
        in_maps.append(
            {
                "xT": xTs[b],
                "wqkv": wqkv_e[hg],
                "wfcT": wfcTs[hg],
                "onesrep": onesrep,
                "triu": triu_f,
            }
        )
    return in_maps


def _is_causal(mask):
    idx = np.arange(mask.shape[0])
    return mask.shape == (L, L) and bool(np.all(mask == (idx[None, :] <= idx[:, None])))


def _reference_numpy(x, mask, w_qkv, w_fc, q_norm_w, k_norm_w, subset_attention_size):
    # slow but general fallback (only used if mask is not causal)
    b, l, d = x.shape
    qkv = x @ w_qkv.T
    q, k, v = np.split(qkv, 3, axis=-1)

    def heads(t):
        return t.reshape(b, l, NH, DH).transpose(0, 2, 1, 3)

    def rms(t, w):
        return t * (1.0 / np.sqrt(np.mean(t * t, -1, keepdims=True) + EPS)) * w

    q, k, v = heads(q), heads(k), heads(v)
    q, k = rms(q, q_norm_w), rms(k, k_norm_w)

    def sdpa(q, k, v, m):
        s = np.einsum("bhqd,bhkd->bhqk", q, k) / np.sqrt(DH)
        s = np.where(m[None, None], s, -1e30)
        s = s - s.max(-1, keepdims=True)
        p = np.exp(s)
        p /= p.sum(-1, keepdims=True)
        return np.einsum("bhqk,bhkd->bhqd", p, v)

    S = int(subset_attention_size) if subset_attention_size is not None else None
    if S is not None and S < l:
        o = np.concatenate(
            [
                sdpa(q[:, :, :S], k[:, :, :S], v[:, :, :S], mask[:S, :S]),
                sdpa(q[:, :, S:], k, v, mask[S:, :]),
            ],
            axis=2,
        )
    else:
        o = sdpa(q, k, v, mask)
    o = o.transpose(0, 2, 1, 3).reshape(b, l, d)
    return (o @ w_fc.T).astype(np.float32)


def kernel(**inputs):
    x = np.asarray(inputs["x"], dtype=np.float32)
    mask = np.asarray(inputs["mask"])
    w_qkv = np.asarray(inputs["w_qkv"], dtype=np.float32)
    w_fc = np.asarray(inputs["w_fc"], dtype=np.float32)
    q_norm_w = np.asarray(inputs["q_norm_w"], dtype=np.float32)
    k_norm_w = np.asarray(inputs["k_norm_w"], dtype=np.float32)

    wqk = q_norm_w * k_norm_w
    if not _is_causal(mask) or np.any(np.abs(wqk) < 1e-20):
        return _reference_numpy(
            x, mask, w_qkv, w_fc, q_norm_w, k_norm_w, inputs.get("subset_attention_size")
        )

    if "nc" not in _CACHE:
        _CACHE["nc"] = _build_nc()
    nc = _CACHE["nc"]

    in_maps = _make_in_maps(x, w_qkv, w_fc, q_norm_w, k_norm_w)
    res = run_bass_kernel_spmd(nc, in_maps, core_ids=list(range(8)))
    parts = [res.results[i]["outp"] for i in range(8)]
    out = np.empty((B, L, D), dtype=np.float32)
    for b in range(B):
        acc = np.zeros((L, D), dtype=np.float64)
        for hg in range(4):
            acc += parts[b * 4 + hg]
        out[b] = acc.astype(np.float32)
    return out
